# revision 1
# baseline (speedup 1.0000x reference)
"""Trainium2 Bass kernel for nn_MultiHeadAttention_73589969649754
(gnn_message_passing / graph cross-attention).

Strategy:
  - Edges sorted by destination node (host-side index prep); each of the
    8 cores owns a contiguous node range per side, split at node
    boundaries so no segment straddles cores.
  - Per core: the 4 input projections are computed as one fused GEMM
    per side (node @ [Wk;Wv].T) over a 2500-node shard, then AllGathered
    (bf16) so every core holds the full [20480, 1024] K|V tables.
  - Edge phase: destination-sorted edges are packed into windows of
    <=128 consecutive nodes / <=1024 edge slots.  Per 128-edge block:
    dma_gather pulls K|V rows, DVE+ACT compute the edge dot product and
    exp (softmax without max-subtraction - safe at these magnitudes),
    and a one-hot matmul on the PE performs the weighted segment-sum
    directly in transposed [channel, node] orientation.  The output
    GEMM (Wo) consumes that orientation with no transposes; the softmax
    denominator z is recovered in the same PSUM pass and divided after
    the GEMM (column scaling commutes).  LeakyReLU+bias on DVE.
  - Host reassembles per-core [512, W*128] outputs by column map.
"""

import math

import numpy as np

N = 20000
E = 160000
C = 512
NCORES = 8
TEMP = float(np.sqrt(C))
NEG = 0.01
NPC = N // NCORES            # 2500 nodes per GEMM shard
NTILES = math.ceil(NPC / 128)
NPAD = NTILES * 128          # 2560 padded shard rows
BLK = 128                    # edges per block
BPW = 8                      # blocks per window
WCAP = BPW * BLK             # 1024 edge slots per window
DUMMY_REL = 999.0
SKIP_AG = False
SKIP_C = False


def _table_row(n):
    return (n // NPC) * NPAD + (n % NPC)


def _prep_side(seg_dst, seg_src):
    seg_dst = np.asarray(seg_dst, np.int64)
    seg_src = np.asarray(seg_src, np.int64)
    perm = np.argsort(seg_dst, kind="stable")
    sd = seg_dst[perm]
    ss = seg_src[perm]

    node_b = [0]
    for c in range(1, NCORES):
        node_b.append(int(sd[min(c * E // NCORES, E - 1)]))
    node_b.append(N)
    for i in range(1, len(node_b)):
        node_b[i] = max(node_b[i], node_b[i - 1])
    edge_b = [int(np.searchsorted(sd, nb, "left")) for nb in node_b]

    deg = np.bincount(sd, minlength=N)

    cores = []
    max_w = 0
    for c in range(NCORES):
        n0, n1 = node_b[c], node_b[c + 1]
        e0 = edge_b[c]
        wins = []
        n, e = n0, e0
        while n < n1:
            wn = we = 0
            while n + wn < n1 and wn < BLK and we + deg[n + wn] <= WCAP:
                we += deg[n + wn]
                wn += 1
            assert wn > 0, "node degree exceeds window capacity"
            wins.append((n, wn, e, we))
            n += wn
            e += we
        assert e == edge_b[c + 1]
        cores.append((n0, n1, wins, sd, ss))
        max_w = max(max_w, len(wins))
    return cores, max_w


def _wrap_idx16(idx_flat):
    """[n] -> [128, n//16] int16, i at [i%16, i//16], replicated x8."""
    n = idx_flat.shape[0]
    a = idx_flat.reshape(n // 16, 16).T.astype(np.int16)
    return np.ascontiguousarray(np.tile(a, (8, 1)))


def _build_core_arrays(cores, W):
    out = []
    for (n0, n1, wins, sd, ss) in cores:
        srcrow = np.zeros((W, WCAP), np.int64)
        dstrow = np.zeros((W, WCAP), np.int64)
        dstrel = np.full((W, WCAP), DUMMY_REL, np.float32)
        colnode = np.full(W * BLK, -1, np.int64)
        for w, (fn, wn, es, ne) in enumerate(wins):
            srcrow[w, :ne] = _table_row(ss[es:es + ne])
            dstrow[w, :ne] = _table_row(sd[es:es + ne])
            dstrel[w, :ne] = (sd[es:es + ne] - fn).astype(np.float32)
            colnode[w * BLK: w * BLK + wn] = np.arange(fn, fn + wn)
        # wrapped int16 index tiles: [128, W*64]
        sidx = np.concatenate(
            [_wrap_idx16(srcrow[w]) for w in range(W)], axis=1)
        didx = np.concatenate(
            [_wrap_idx16(dstrow[w]) for w in range(W)], axis=1)
        # dstrel as [128, W*8]: [p, w*8+b] = rel of edge b*128+p in window w
        drel = np.ascontiguousarray(
            dstrel.reshape(W, BPW, BLK).transpose(2, 0, 1).reshape(BLK, W * BPW))
        out.append(dict(sidx=sidx, didx=didx, drel=drel, colnode=colnode))
    return out


def _build_program(W):
    import concourse.bacc as bacc
    import concourse.tile as tile
    from concourse import mybir

    dt = mybir.dt
    f32, bf16, i16 = dt.float32, dt.bfloat16, dt.int16
    AF = mybir.ActivationFunctionType
    OP = mybir.AluOpType

    nc = bacc.Bacc("TRN2", target_bir_lowering=False, debug=False,
                   enable_asserts=True, num_devices=NCORES)

    # ---- I/O ----
    nT_in = {s: nc.dram_tensor(f"nT_{s}", [C, NPAD], bf16,
                               kind="ExternalInput").ap() for s in "LR"}
    wkvT = nc.dram_tensor("wkvT", [128, 4 * 1024], bf16,
                          kind="ExternalInput").ap()
    woT = nc.dram_tensor("woT", [128, 4 * 512], bf16,
                         kind="ExternalInput").ap()
    bo_in = nc.dram_tensor("bo", [128, 4], f32, kind="ExternalInput").ap()
    iota_in = nc.dram_tensor("iota", [128, 128], f32,
                             kind="ExternalInput").ap()
    sidx_in = {s: nc.dram_tensor(f"sidx_{s}", [128, W * 64], i16,
                                 kind="ExternalInput").ap() for s in "LR"}
    didx_in = {s: nc.dram_tensor(f"didx_{s}", [128, W * 64], i16,
                                 kind="ExternalInput").ap() for s in "LR"}
    drel_in = {s: nc.dram_tensor(f"drel_{s}", [128, W * BPW], f32,
                                 kind="ExternalInput").ap() for s in "LR"}
    hT_out = {s: nc.dram_tensor(f"hT_{s}", [C, W * BLK], f32,
                                kind="ExternalOutput").ap() for s in "LR"}

    # ---- internal DRAM ----
    tkv_sh = {s: nc.dram_tensor(f"tkv_sh_{s}", [NPAD, 2 * C], bf16).ap()
              for s in "LR"}
    shared = "Shared" if NCORES > 4 else "Local"
    tkv = {s: nc.dram_tensor(f"tkv_{s}", [NCORES * NPAD, 2 * C], bf16,
                             addr_space=shared).ap() for s in "LR"}

    with tile.TileContext(nc) as tc:
        with tc.tile_pool(name="const", bufs=1) as cpool:
            # constants
            wkvT_sb = cpool.tile([128, 4 * 1024], bf16)
            nc.sync.dma_start(wkvT_sb[:], wkvT[:, :])
            woT_sb = cpool.tile([128, 4 * 512], bf16)
            nc.sync.dma_start(woT_sb[:], woT[:, :])
            bo_sb = cpool.tile([128, 4], f32)
            nc.sync.dma_start(bo_sb[:], bo_in[:, :])
            iota_sb = cpool.tile([128, 128], f32)
            nc.sync.dma_start(iota_sb[:], iota_in[:, :])
            ones_col = cpool.tile([128, 1], bf16)
            nc.vector.memset(ones_col[:], 1.0)
            ones_row = cpool.tile([1, 128], bf16)
            nc.vector.memset(ones_row[:], 1.0)
            idx_sb = {}
            for s in "LR":
                sidx_sb = cpool.tile([128, W * 64], i16, tag=f"sidx{s}")
                nc.sync.dma_start(sidx_sb[:], sidx_in[s][:, :])
                didx_sb = cpool.tile([128, W * 64], i16, tag=f"didx{s}")
                nc.sync.dma_start(didx_sb[:], didx_in[s][:, :])
                drel_sb = cpool.tile([128, W * BPW], f32, tag=f"drel{s}")
                nc.sync.dma_start(drel_sb[:], drel_in[s][:, :])
                idx_sb[s] = (sidx_sb, didx_sb, drel_sb)

            # ---- phase A: projection GEMMs into table shards ----
            with (
                tc.tile_pool(name="feat", bufs=1) as fpool,
                tc.tile_pool(name="gemm_sb", bufs=3) as gsb,
                tc.tile_pool(name="psum_gemm", bufs=2, space="PSUM") as pg,
            ):
                for s in "LR":
                    feat = []
                    for cc in range(4):
                        t = fpool.tile([128, NPAD], bf16, tag=f"feat{s}{cc}")
                        nc.sync.dma_start(
                            t[:], nT_in[s][cc * 128:(cc + 1) * 128, :])
                        feat.append(t)
                    for ti in range(NTILES):
                        sb = gsb.tile([128, 1024], bf16)
                        for half in range(2):
                            ps = pg.tile([128, 512], f32)
                            for cc in range(4):
                                nc.tensor.matmul(
                                    ps[:],
                                    lhsT=feat[cc][:, ti * 128:(ti + 1) * 128],
                                    rhs=wkvT_sb[:, cc * 1024 + half * 512:
                                                cc * 1024 + half * 512 + 512],
                                    start=(cc == 0), stop=(cc == 3))
                            nc.scalar.copy(
                                sb[:, half * 512:(half + 1) * 512], ps[:])
                        nc.sync.dma_start(
                            tkv_sh[s][ti * 128:(ti + 1) * 128, :], sb[:])

                # ---- phase B: AllGather both tables ----
                if not SKIP_AG:
                    for s in "LR":
                        nc.gpsimd.collective_compute(
                            "AllGather", mybir.AluOpType.bypass,
                            replica_groups=[list(range(NCORES))],
                            ins=[tkv_sh[s]], outs=[tkv[s]])

            # ---- phase C: edge processing ----
            with (
                tc.tile_pool(name="gath", bufs=3) as gpool,
                tc.tile_pool(name="blk", bufs=4) as sp,
                tc.tile_pool(name="ohs", bufs=2 * BPW) as ohpool,
                tc.tile_pool(name="tail", bufs=3) as tp,
                tc.tile_pool(name="pmsg", bufs=2, space="PSUM") as pmsg,
                tc.tile_pool(name="pz", bufs=2, space="PSUM") as pz,
                tc.tile_pool(name="pzbc", bufs=2, space="PSUM") as pzbc,
                tc.tile_pool(name="ph", bufs=2, space="PSUM") as ph,
            ):
                nidx_reg = nc.gpsimd.to_reg(WCAP)
                for s, o in ((() if SKIP_C else (("L", "R"), ("R", "L")))):
                    sidx_sb, didx_sb, drel_sb = idx_sb[s]
                    hacc = cpool.tile([128, 4 * W * 128], f32, tag=f"hacc{s}")
                    for w in range(W):
                        kv = gpool.tile([128, BPW, 2 * C], bf16, tag="kv")
                        nc.gpsimd.dma_gather(
                            kv[:], tkv[o][:, :], sidx_sb[:, w * 64:(w + 1) * 64],
                            WCAP, nidx_reg, 2 * C)
                        kd = gpool.tile([128, BPW, C], bf16, tag="kd")
                        nc.gpsimd.dma_gather(
                            kd[:], tkv[s][:, 0:C], didx_sb[:, w * 64:(w + 1) * 64],
                            WCAP, nidx_reg, C, elem_step=2 * C)

                        msgT_ps = pmsg.tile([128, 512], f32)
                        z_ps = pz.tile([1, 128], f32)
                        sacc = sp.tile([128, BPW], f32, tag="sacc")
                        for b in range(BPW):
                            prod = sp.tile([128, C], bf16, tag="prod")
                            nc.vector.tensor_tensor(
                                prod[:], kv[:, b, 0:C], kd[:, b, :], op=OP.mult)
                            nc.scalar.activation(
                                prod[:], prod[:], AF.Copy, bias=0.0,
                                scale=1.0, accum_out=sacc[:, b:b + 1])
                        eh = sp.tile([128, BPW], f32, tag="eh")
                        nc.scalar.activation(eh[:], sacc[:], AF.Exp,
                                             scale=1.0 / TEMP)
                        ohs = []
                        for b in range(BPW):
                            oh = ohpool.tile([128, 128], bf16, tag="oh")
                            nc.vector.tensor_scalar(
                                oh[:], iota_sb[:],
                                drel_sb[:, w * BPW + b: w * BPW + b + 1],
                                eh[:, b:b + 1], op0=OP.is_equal, op1=OP.mult)
                            ohs.append(oh)
                        for cc in range(4):
                            for b in range(BPW):
                                nc.tensor.matmul(
                                    msgT_ps[:, cc * 128:(cc + 1) * 128],
                                    lhsT=kv[:, b, C + cc * 128: C + (cc + 1) * 128],
                                    rhs=ohs[b][:],
                                    start=(b == 0), stop=(b == BPW - 1))
                        for b in range(BPW):
                            nc.tensor.matmul(
                                z_ps[:], lhsT=ones_col[:], rhs=ohs[b][:],
                                start=(b == 0), stop=(b == BPW - 1))

                        # window tail
                        zm = tp.tile([1, 128], f32, tag="zm")
                        nc.vector.tensor_scalar_max(zm[:], z_ps[:], 1e-30)
                        zr = tp.tile([1, 128], f32, tag="zr")
                        nc.vector.reciprocal(zr[:], zm[:])
                        zrb = tp.tile([1, 128], bf16, tag="zrb")
                        nc.vector.tensor_copy(zrb[:], zr[:])
                        zbc_ps = pzbc.tile([128, 128], f32)
                        nc.tensor.matmul(zbc_ps[:], lhsT=ones_row[:], rhs=zrb[:],
                                         start=True, stop=True)
                        zbc = tp.tile([128, 128], f32, tag="zbc")
                        nc.vector.tensor_copy(zbc[:], zbc_ps[:])
                        msgT_sb = tp.tile([128, 512], bf16, tag="msgT")
                        for nch in range(4):
                            nc.vector.tensor_tensor(
                                msgT_sb[:, nch * 128:(nch + 1) * 128],
                                msgT_ps[:, nch * 128:(nch + 1) * 128],
                                zbc[:], op=OP.mult)
                        hT_ps = ph.tile([128, 512], f32)
                        for oc in range(4):
                            for cc in range(4):
                                nc.tensor.matmul(
                                    hT_ps[:, oc * 128:(oc + 1) * 128],
                                    lhsT=woT_sb[:, cc * 512 + oc * 128:
                                                cc * 512 + oc * 128 + 128],
                                    rhs=msgT_sb[:, cc * 128:(cc + 1) * 128],
                                    start=(cc == 0), stop=(cc == 3))
                        for oc in range(4):
                            x = hacc[:, (oc * W + w) * 128:
                                     (oc * W + w) * 128 + 128]
                            nc.scalar.activation(
                                x, hT_ps[:, oc * 128:(oc + 1) * 128],
                                AF.Identity, bias=bo_sb[:, oc:oc + 1])
                            x2 = tp.tile([128, 128], f32, tag="x2")
                            nc.vector.tensor_scalar_mul(x2[:], x, NEG)
                            nc.vector.tensor_tensor(x, x, x2[:], op=OP.max)
                    for oc in range(4):
                        nc.sync.dma_start(
                            hT_out[s][oc * 128:(oc + 1) * 128, :],
                            hacc[:, oc * W * 128:(oc + 1) * W * 128])
    nc.compile()
    return nc


def _host_inputs(inputs):
    import ml_dtypes
    bf16 = ml_dtypes.bfloat16

    nl = np.asarray(inputs["node_left"], np.float32)
    nr = np.asarray(inputs["node_right"], np.float32)
    Wk = np.asarray(inputs["Wk"], np.float32)
    Wv = np.asarray(inputs["Wv"], np.float32)
    Wo = np.asarray(inputs["Wo"], np.float32)
    bo = np.asarray(inputs["bo"], np.float32)
    sl = np.asarray(inputs["segmentation_index_left"], np.int64)
    sr = np.asarray(inputs["segmentation_index_right"], np.int64)

    coresL, wL = _prep_side(sl, sr)
    coresR, wR = _prep_side(sr, sl)
    W = max(wL, wR)
    arrL = _build_core_arrays(coresL, W)
    arrR = _build_core_arrays(coresR, W)

    Wkv = np.concatenate([Wk, Wv], 0)               # [1024, 512]
    WkvT = Wkv.T                                    # [512, 1024]
    wkvT_arr = np.zeros((128, 4 * 1024), np.float32)
    for cc in range(4):
        wkvT_arr[:, cc * 1024:(cc + 1) * 1024] = \
            WkvT[cc * 128:(cc + 1) * 128, :]
    woT_arr = np.zeros((128, 4 * 512), np.float32)
    for cc in range(4):
        for oc in range(4):
            woT_arr[:, cc * 512 + oc * 128: cc * 512 + (oc + 1) * 128] = \
                Wo[oc * 128:(oc + 1) * 128, cc * 128:(cc + 1) * 128].T
    bo_arr = bo.reshape(4, 128).T.copy()            # [128, 4]
    iota_arr = np.broadcast_to(
        np.arange(128, dtype=np.float32)[None, :], (128, 128)).copy()

    def shardT(feat, c):
        sh = np.zeros((C, NPAD), np.float32)
        sh[:, :NPC] = feat[c * NPC:(c + 1) * NPC].T
        return np.ascontiguousarray(sh).astype(bf16)

    in_maps = []
    for c in range(NCORES):
        in_maps.append({
            "nT_L": shardT(nl, c),
            "nT_R": shardT(nr, c),
            "wkvT": wkvT_arr.astype(bf16),
            "woT": woT_arr.astype(bf16),
            "bo": bo_arr,
            "iota": iota_arr,
            "sidx_L": arrL[c]["sidx"],
            "didx_L": arrL[c]["didx"],
            "drel_L": arrL[c]["drel"],
            "sidx_R": arrR[c]["sidx"],
            "didx_R": arrR[c]["didx"],
            "drel_R": arrR[c]["drel"],
        })
    return in_maps, arrL, arrR, W


def _assemble(results, arrs, key):
    out = np.zeros((N, C), np.float32)
    for c in range(NCORES):
        hT = np.asarray(results[c][key], np.float32)
        cn = arrs[c]["colnode"]
        m = cn >= 0
        out[cn[m]] = hT[:, m].T
    return out


_RUN_KWARGS = {}


def kernel(**inputs):
    from concourse.bass_utils import run_bass_kernel_spmd

    in_maps, arrL, arrR, W = _host_inputs(inputs)
    nc = _build_program(W)
    res = run_bass_kernel_spmd(nc, in_maps, core_ids=list(range(NCORES)),
                               **_RUN_KWARGS)
    out_l = _assemble(res.results, arrL, "hT_L")
    out_r = _assemble(res.results, arrR, "hT_R")
    kernel.last_results = res
    kernel.last_nc = nc
    kernel.last_W = W
    return (out_l, out_r)

